# revision 68
# baseline (speedup 1.0000x reference)
"""LowPassMSELoss Trainium2 kernel.

Math: loss = mean((lfilter(b,a,o) - lfilter(b,a,t))^2)
    = mean(lfilter(b,a,o-t)^2)               [filter is linear]
    = mean(conv(o-t, h)^2)                   [h = impulse response, truncated
                                              to K=128 taps; max pole radius
                                              0.869 -> tail < 2e-8]

Data-parallel over 8 cores: 2 rows of T=262144 per core, host sums the
per-core partial sums.

Per-core pipeline (all post-subtract data in bf16):
  - input rows arrive as chunked Pool-engine (SWDGE) cast DMAs
    fp32->bf16, f-major layout [128 partitions x (2 signals) x f]
    (partition p = contiguous 2048-sample segment)
  - a tiny SP DMA fetches the 128 "history" samples preceding each
    partition's segment (ext), so every 128-col block of the transposed
    stream has its predecessor locally: no wrap-around dependency on the
    end of the row
  - d = o - t on DVE (bf16 2x mode)
  - PE-transposes 128x128 blocks of d into Xb[i, 128*u + p] = d[p, 128*u+i]
    (u=0 is the ext block), 17 blocks per row, batched 5/4/4/4 per PSUM
    bank, one DVE copy per batch
  - conv tile j (j=0..3): psum[jj, n] = sum_i A[i,jj] Xb[i, 128+512j+n]
                                       + sum_i B[i,jj] Xb[i, 512j+n]
    (two accumulating bf16 matmuls, Toeplitz lhsT built host-side from h)
  - ACT squares two adjacent conv psum tiles at once ([128,1024]) with
    accum_out -> per-partition partial sums
  - host: sum partials over 8 cores / (16*262144)
"""

import os
import numpy as np

B, T = 16, 262144
NCORES = 8
ROWS_PER_CORE = B // NCORES          # 2
F = 2048                             # f-cols of natural layout (T / 128)
K = 128                              # FIR taps
NJ = F // 512                        # 4 conv output tiles per row
NU = 17                              # transpose blocks per row (ext + 16)
DW = K + F                           # d/Xb width (128 ext cols + data)

# chunk ranges per row in d-col space (x = sample offset + 128; the host
# prepends 128 zeros per row so x maps to padded-HBM offset 2048p + x
# directly); last chunk small to shrink the post-stream tail
CHUNKS = [(0, 1152), (1152, 1920), (1920, 2176)]
# transpose groups (u-blocks), aligned so conv j only needs groups 0..j-ish
# and only the tiny last group depends on the last chunk
TGROUPS = [(0, 5), (5, 9), (9, 13), (13, 15), (15, 17)]
# DMA issue order over (row, chunk)
ARRIVALS = [(0, 0), (1, 0), (0, 1), (0, 2), (1, 1), (1, 2)]

last_exec_time_ns = None
_CACHE = {}


def _np_bf16():
    import ml_dtypes

    return np.dtype(ml_dtypes.bfloat16)


def _impulse_response(b, a, n):
    """First n samples of the IIR impulse response, float64, DF2T like scipy."""
    b = np.asarray(b, np.float64)
    a = np.asarray(a, np.float64)
    b = b / a[0]
    a = a / a[0]
    order = len(a) - 1
    z = np.zeros(order, np.float64)
    h = np.empty(n, np.float64)
    for i in range(n):
        x = 1.0 if i == 0 else 0.0
        y = b[0] * x + z[0]
        znew = np.empty(order, np.float64)
        znew[: order - 1] = z[1:] + b[1:order] * x - a[1:order] * y
        znew[order - 1] = b[order] * x - a[order] * y
        z = znew
        h[i] = y
    return h


def _toeplitz_lhsts(h):
    """lhsT_A[i,j] = h[j-i] (j>=i), lhsT_B[i,j] = h[128+j-i] (i>j).

    y[128n+j] = sum_{i<=j} h[j-i]*cur[i] + sum_{i>j} h[128+j-i]*prev[i]
    matmul(out, lhsT, rhs): out[j, n] = sum_i lhsT[i, j] * rhs[i, n]
    """
    i = np.arange(K)[:, None]
    j = np.arange(K)[None, :]
    dj = j - i
    A = np.where(dj >= 0, h[np.clip(dj, 0, K - 1)], 0.0)
    Bm = np.where(dj < 0, h[np.clip(K + dj, 0, K - 1)], 0.0)
    bf16 = _np_bf16()
    return A.astype(bf16), Bm.astype(bf16)


def _prune_final_drain(nc):
    """The closing SP Drain waits every DMA/engine semaphore Tile saw, but
    every one of them is transitively implied by the output DMA's completion:
    the partials DMA waits the final Activation, which (by per-engine queue
    order and the dataflow into out_sb) postdates all compute and all input
    DMA consumers.  Keep only the output DMA's own sem wait so the epilogue
    doesn't serialize a dozen 100ns sem checks."""
    out_sems = set()
    for f in nc.m.functions:
        for bb in f.blocks:
            for ins in bb.instructions:
                if type(ins).__name__ == "InstDMACopy":
                    outs = getattr(ins, "outs", None) or []
                    is_out = any(
                        "partials" in str(getattr(o, "tensor_name", "") or o)
                        for o in outs
                    )
                    if is_out and ins.sync_info:
                        for u in ins.sync_info.on_update or []:
                            nm = getattr(u, "ant_name", "") or ""
                            if nm:
                                out_sems.add(nm)
    if not out_sems:
        return
    for f in nc.m.functions:
        for bb in f.blocks:
            for ins in bb.instructions:
                if "Drain" not in type(ins).__name__:
                    continue
                si = ins.sync_info
                if si is None or not si.on_wait or len(si.on_wait) < 3:
                    continue
                waits = list(si.on_wait)
                names = {getattr(w, "ant_name", "") or "" for w in waits}
                if not (names & out_sems):
                    continue
                si.on_wait = [
                    w for w in waits if (getattr(w, "ant_name", "") or "") in out_sems
                ]


def _drop_vacuous_self_waits(nc):
    """trn2 codegen allows one sync-wait per instruction; Tile sometimes
    attaches several.  Pass 1 drops same-engine self-waits whose threshold
    is already guaranteed by queue position (engine queues issue in order
    and every same-engine op increments the engine sem; all our PE ops are
    matmuls — completion is pc-monotone — and ACT/DVE/Pool execute ops
    serially, so position implies completion for same-engine hazards here).
    Pass 2 hoists any remaining extra waits onto preceding same-engine
    no-ops so every instruction carries at most one wait."""
    from concourse import mybir

    prior_incs = {}
    for f in nc.m.functions:
        for bb in f.blocks:
            for ins in bb.instructions:
                si = ins.sync_info
                if si is None:
                    continue
                waits = list(si.on_wait or [])
                if len(waits) > 1:
                    kept = []
                    for w in waits:
                        name = getattr(w, "ant_name", "") or ""
                        eng = getattr(getattr(ins, "engine", None), "value", "zz")
                        if (
                            name.startswith(eng)
                            and prior_incs.get(name, 0) >= (w.wait_value or 0)
                        ):
                            continue
                        kept.append(w)
                    si.on_wait = kept
                for u in si.on_update or []:
                    name = getattr(u, "ant_name", "") or ""
                    if name:
                        prior_incs[name] = prior_incs.get(name, 0) + (
                            u.update_value or 1
                        )
    nsplit = 0
    for f in nc.m.functions:
        for bb in f.blocks:
            new_list = []
            for ins in bb.instructions:
                si = ins.sync_info
                if si is not None and si.on_wait and len(si.on_wait) > 1:
                    waits = list(si.on_wait)
                    for w in waits[:-1]:
                        nop = mybir.InstNoOp(name=f"{ins.name}-wait{nsplit}")
                        nsplit += 1
                        nop.engine = ins.engine
                        nop.sync_info = mybir.SyncInfo(on_wait=[w], on_update=[])
                        new_list.append(nop)
                    si.on_wait = [waits[-1]]
                new_list.append(ins)
            bb.instructions = new_list


def _build_bass():
    import concourse.bass as bass
    import concourse.tile as tile
    from concourse import mybir

    dt = mybir.dt
    nc = bass.Bass(trn_type="TRN2")

    # host prepends K zeros per row: ot[r, s, K + t] = signal sample t
    ot_h = nc.dram_tensor(
        "ot", [ROWS_PER_CORE, 2, K + T], dt.float32, kind="ExternalInput"
    )
    # host-packed [p, c, j] so each partition's row is one contiguous 768B run
    C_h = nc.dram_tensor("consts", [K, 3, K], dt.bfloat16, kind="ExternalInput")
    out_h = nc.dram_tensor(
        "partials", [128, ROWS_PER_CORE * 2], dt.float32, kind="ExternalOutput"
    )

    R = ROWS_PER_CORE
    with tile.TileContext(nc) as tc:
        with (
            tc.tile_pool(name="consts", bufs=1) as consts,
            tc.tile_pool(name="warm", bufs=1) as warm_pool,
            tc.tile_pool(name="io", bufs=R) as io_pool,
            tc.tile_pool(name="dpool", bufs=R) as dpool,
            tc.tile_pool(name="xb", bufs=R) as xbpool,
            tc.tile_pool(name="ptr", bufs=4, space="PSUM") as ptr_pool,
            tc.tile_pool(name="pconv", bufs=2, space="PSUM") as pconv_pool,
            tc.tile_pool(name="scr", bufs=2) as scr_pool,
            tc.tile_pool(name="outp", bufs=1) as out_pool,
        ):
            # ---- warmup: ramp the PE clock + load ACT Square table while
            # the first input DMAs are in flight ----
            wz = warm_pool.tile([128, 8 + 512], dt.bfloat16, tag="wz")
            nc.vector.memset(wz[:], 0.0)
            # warmup psum borrows a transpose-pool slot (freed well before
            # the first transposes need it)
            wp = ptr_pool.tile([8, 256], dt.float32, tag="tr", name="wp")
            for _ in range(5):
                nc.tensor.matmul(
                    wp[:], wz[:, 0:8], wz[:, 8:264], start=True, stop=True
                )
            wact = warm_pool.tile([128, 1], dt.float32, tag="wact")
            nc.scalar.activation(
                wact[:], wz[:, 0:1], mybir.ActivationFunctionType.Square
            )

            # ---- consts (SP queue, bf16, tiny) ----
            c_raw = consts.tile([K, 3, K], dt.bfloat16, tag="Craw")
            nc.sync.dma_start(c_raw[:], C_h[:])
            # funnel the const-DMA dep through DVE so PE ops wait on one engine
            c_sb = consts.tile([K, 3, K], dt.bfloat16, tag="C")
            nc.vector.tensor_copy(c_sb[:], c_raw[:])
            A_sb = c_sb[:, 0, :]
            B_sb = c_sb[:, 1, :]
            I_sb = c_sb[:, 2, :]

            out_sb = out_pool.tile([128, R * 2], dt.float32)

            # ---- staggered row pipelines: row 0's chunks are prioritized in
            # the DMA stream so its whole pipeline (through its last square)
            # hides under row 1's loads; emission follows arrival order so no
            # engine queue is head-of-line blocked ----
            AP = type(ot_h[:])
            ot_sbs = [
                io_pool.tile([128, 2, DW], dt.bfloat16, tag="ot", name=f"ot_sb{r}")
                for r in range(R)
            ]
            d_sbs = [
                dpool.tile([128, DW], dt.bfloat16, tag="d", name=f"d_sb{r}")
                for r in range(R)
            ]
            xbs = [
                xbpool.tile([128, DW], dt.bfloat16, tag="xb", name=f"xb{r}")
                for r in range(R)
            ]
            pys = [[None, None] for _ in range(R)]
            next_group = [0] * R
            next_conv = [0] * R
            next_sq = [0] * R

            # arrival order: both rows' first chunks, then all of row 0,
            # then the rest of row 1
            arrivals = ARRIVALS
            for r, ci in arrivals:
                x0, x1 = CHUNKS[ci]
                # src[p, s, x] = ot[r, s, 2048p + x0 + x]: overlapping
                # 2176-wide windows on a 2048 partition stride
                src = AP(
                    ot_h[:].tensor,
                    r * 2 * (K + T) + x0,
                    [[F, 128], [K + T, 2], [1, x1 - x0]],
                )
                nc.gpsimd.dma_start(ot_sbs[r][:, :, x0:x1], src)

            def emit_group(r, g):
                u0, u1 = TGROUPS[g]
                ptr = ptr_pool.tile(
                    [128, 640], dt.bfloat16, tag="tr", name=f"ptr{r}_{g}"
                )
                for q, u in enumerate(range(u0, u1)):
                    nc.tensor.transpose(
                        ptr[:, 128 * q : 128 * (q + 1)],
                        d_sbs[r][:, 128 * u : 128 * (u + 1)],
                        I_sb[:],
                    )
                w = 128 * (u1 - u0)
                # GPSIMD/Pool cannot read PSUM on real HW, so all PSUM->SBUF
                # copies stay on DVE
                nc.vector.tensor_copy(xbs[r][:, 128 * u0 : 128 * u1], ptr[:, 0:w])

            def emit_conv(r, j):
                half, jj = divmod(j, 2)
                if jj == 0:
                    pys[r][half] = pconv_pool.tile(
                        [128, 1024], dt.float32, tag="y", name=f"py{r}_{half}"
                    )
                dst = pys[r][half][:, 512 * jj : 512 * (jj + 1)]
                xb = xbs[r]
                nc.tensor.matmul(
                    dst, A_sb[:], xb[:, K + 512 * j : K + 512 * (j + 1)],
                    start=True, stop=False,
                )
                nc.tensor.matmul(
                    dst, B_sb[:], xb[:, 512 * j : 512 * (j + 1)],
                    start=False, stop=True,
                )

            def emit_square(r, half):
                # paired squares: half 0 -> (j0,j1), half 1 -> (j2,j3)
                acc = out_sb[:, 2 * r + half : 2 * r + half + 1]
                scr = scr_pool.tile(
                    [128, 1024], dt.bfloat16, tag="scr", name=f"scr{r}_{half}"
                )
                nc.scalar.activation(
                    scr[:], pys[r][half][:], mybir.ActivationFunctionType.Square,
                    accum_out=acc,
                )

            def emit_sub(r, ci):
                x0, x1 = CHUNKS[ci]
                nc.vector.tensor_sub(
                    d_sbs[r][:, x0:x1],
                    ot_sbs[r][:, 0, x0:x1],
                    ot_sbs[r][:, 1, x0:x1],
                )

            # ---- static list schedule: estimate each chunk's DMA completion
            # from the cost model (serial Pool DGE, serial DMA device), then
            # emit every op in estimated-readiness order so no engine queue
            # head-of-line blocks another row's earlier-ready work ----
            DGE, DMA_DELAY, SEM = 1142.0, 650.0, 900.0
            dev_free = 2100.0  # c_raw transfer done on the shared DMA device
            fin = {}
            for i, (r, ci) in enumerate(arrivals):
                x0, x1 = CHUNKS[ci]
                dge_done = 100.0 + (i + 1) * DGE
                start = max(dge_done + DMA_DELAY, dev_free)
                dev_free = start + 1.422 * (x1 - x0)
                fin[(r, ci)] = dev_free + SEM

            chunk_of_x = lambda r, x: fin[
                (r, next(ci for ci, (a, b) in enumerate(CHUNKS) if x <= b))
            ]
            ops = []
            for r in range(R):
                for ci in range(len(CHUNKS)):
                    ops.append((fin[(r, ci)], 0, "sub", r, ci))
                for g, (u0, u1) in enumerate(TGROUPS):
                    ops.append((chunk_of_x(r, 128 * u1) + 300, 1, "grp", r, g))
                for j in range(NJ):
                    ops.append((chunk_of_x(r, 640 + 512 * j) + 700, 2, "conv", r, j))
                for half in range(2):
                    ops.append(
                        (chunk_of_x(r, 640 + 512 * (2 * half + 1)) + 1100, 3, "sq", r, half)
                    )
            ops.sort(key=lambda t: (t[0], t[1]))
            for _, _, kind, r, idx in ops:
                if kind == "sub":
                    emit_sub(r, idx)
                elif kind == "grp":
                    emit_group(r, idx)
                elif kind == "conv":
                    emit_conv(r, idx)
                else:
                    emit_square(r, idx)

            nc.sync.dma_start(out_h[:], out_sb[:])

    _prune_final_drain(nc)
    _drop_vacuous_self_waits(nc)
    return nc


def _consts_np(b, a):
    h = _impulse_response(np.asarray(b, np.float64), np.asarray(a, np.float64), K)
    A_m, B_m = _toeplitz_lhsts(h)
    bf16 = _np_bf16()
    # packed [p, c, j]: each partition's 3*128 values are one contiguous run
    return np.ascontiguousarray(
        np.stack([A_m, B_m, np.eye(K, dtype=bf16)]).transpose(1, 0, 2)
    )


def _padded_ot(output, target):
    ot = np.zeros((output.shape[0], 2, K + T), np.float32)
    ot[:, 0, K:] = output
    ot[:, 1, K:] = target
    return ot


def kernel(output, target, b, a):
    global last_exec_time_ns
    from concourse.bass_utils import run_bass_kernel_spmd

    output = np.asarray(output, np.float32)
    target = np.asarray(target, np.float32)

    if "nc" not in _CACHE:
        _CACHE["nc"] = _build_bass()
    nc = _CACHE["nc"]

    consts = _consts_np(b, a)

    # [B, 2, K+T]: 128 zeros of "history" prepended per row so the kernel's
    # overlapping windows never go out of bounds
    ot = _padded_ot(output, target)
    in_maps = []
    for c in range(NCORES):
        rows = slice(c * ROWS_PER_CORE, (c + 1) * ROWS_PER_CORE)
        in_maps.append(
            {
                "ot": np.ascontiguousarray(ot[rows]),
                "consts": consts,
            }
        )

    res = run_bass_kernel_spmd(
        nc,
        in_maps,
        core_ids=list(range(NCORES)),
        trace=bool(int(os.environ.get("LP_TRACE", "0"))),
    )
    last_exec_time_ns = res.exec_time_ns

    total = np.float64(0.0)
    for r in res.results:
        total += r["partials"].astype(np.float64).sum()
    return np.float32(total / (B * T))
